# revision 1
# baseline (speedup 1.0000x reference)
"""Trainium2 Bass kernel for nn_CascadedSpatialCrossAttention.

Sharding: data-parallel over batch. B=8 batch elements -> 8 NeuronCores,
one batch element per core. Params are replicated. No collectives.

Per-core layout: an image tensor (64ch, 128, 128) is stored with
partition p = c + 64*parity (parity = h % 2), i.e. even rows of channel c
on partition c, odd rows on partition c+64.  This uses all 128 partitions
for elementwise/reduce work and lets conv3x3 taps be K-stacked in pairs
(even+odd source rows share one free-dim offset).

feat tile is [128, 66, 130]: 64 row-pairs + 2 halo rows, 128 cols + 2 pad
cols, so all 9 conv taps read in-bounds (halo/pad are zero).

All large matmuls run in fp32r (TF32) mode: 1 PE cycle/row instead of 4
for fp32.  The BIR verifier requires every producer of an fp32r-matmul
input to declare an fp32r output, so those producers write through
.bitcast(float32r) views (the engine rounds on write).

Schedule: per group, the PE conv stream is emitted first (it only needs
feat), the pooled-attention/gating chain runs concurrently on DVE/Act/
Pool, conv results are evicted to x12 chunk-by-chunk on Pool, and the
weights phase streams per-chunk through PE->Act->DVE.  The next group's
input tile is prefetched into t_pf during compute.  rsqrt is computed
with a Newton iteration on DVE so the Act engine never switches
activation tables (sigmoid/square/copy share one set).
"""

import sys

sys.path.insert(0, "/opt/trn_rl_repo")

import numpy as np

import concourse.bass as bass
import concourse.bacc as bacc
import concourse.tile as tile
from concourse import mybir
from concourse.masks import make_identity

F32 = mybir.dt.float32
F32R = mybir.dt.float32r
I32 = mybir.dt.int32
AF = mybir.ActivationFunctionType
ALU = mybir.AluOpType
AX = mybir.AxisListType

G = 4          # groups
C = 64         # channels per group
H = W = 128
J = 64         # row pairs per parity
NCHUNK = 16    # free-dim chunks of 512 (4 rows) per parity
EPS = 1e-5


def _r(ap):
    # fp32 -> fp32r view: 1 cycle/row matmul (vs 4 for fp32) at free >= 256
    return ap.bitcast(F32R)


def _sigmoid_softmax(nc, sm, vec, n):
    """softmax over vec [1, n] (SBUF, partition 0), using sigmoid-based exp
    (exp(z) = s/(1-s), s = sigmoid(z), z <= 0 after max subtraction).
    Returns an AP [1, n] with the softmax result (a fresh tile from pool sm).
    Only uses the Sigmoid table set (no Exp load)."""
    mx = sm.tile([1, 1], F32, tag="sm_mx")
    nc.vector.tensor_reduce(mx, vec, axis=AX.X, op=ALU.max)
    z = sm.tile([1, n], F32, tag="sm_z")
    nc.vector.tensor_scalar(z, vec, mx, None, op0=ALU.subtract)
    s = sm.tile([1, n], F32, tag="sm_s")
    nc.scalar.activation(s, z, AF.Sigmoid)
    u = sm.tile([1, n], F32, tag="sm_u")
    # u = 1 - s  ==  (s * -1) + 1
    nc.vector.tensor_scalar(u, s, -1.0, 1.0, op0=ALU.mult, op1=ALU.add)
    r = sm.tile([1, n], F32, tag="sm_r")
    nc.vector.reciprocal(r, u)
    e = sm.tile([1, n], F32, tag="sm_e")
    nc.vector.tensor_tensor(e, s, r, op=ALU.mult)
    se = sm.tile([1, 1], F32, tag="sm_se")
    nc.vector.tensor_reduce(se, e, axis=AX.X, op=ALU.add)
    rs = sm.tile([1, 1], F32, tag="sm_rs")
    nc.vector.reciprocal(rs, se)
    out = sm.tile([1, n], F32, tag="sm_out")
    nc.vector.tensor_scalar(out, e, rs, None, op0=ALU.mult)
    return out


def _rsqrt(nc, sm, var_ap, n, tag):
    """1/sqrt(var + EPS) on DVE only (quake initial guess + 2 Newton
    iterations) - avoids Act Sqrt table loads."""
    z = sm.tile([1, n], F32, tag=tag + "z")
    nc.vector.tensor_scalar(z, var_ap, EPS, None, op0=ALU.add)
    t = sm.tile([1, n], I32, tag=tag + "t")
    nc.vector.tensor_scalar(t, z.bitcast(I32), 1, None,
                            op0=ALU.logical_shift_right)
    y = sm.tile([1, n], F32, tag=tag + "y")
    # 0x5f3759df - t == (t ^ -1) + 0x5f3759e0  (bitwise and arith ops
    # cannot share one tensor_scalar on HW)
    nc.vector.tensor_scalar(t, t, -1, None, op0=ALU.bitwise_xor)
    nc.vector.tensor_scalar(y.bitcast(I32), t, 0x5F3759E0, None, op0=ALU.add)
    a = sm.tile([1, n], F32, tag=tag + "a")
    for _ in range(1):
        nc.vector.tensor_tensor(a, z, y, op=ALU.mult)
        nc.vector.tensor_tensor(a, a, y, op=ALU.mult)
        nc.vector.tensor_scalar(a, a, -0.5, 1.5, op0=ALU.mult, op1=ALU.add)
        nc.vector.tensor_tensor(y, y, a, op=ALU.mult)
    return y


def _col_to_row(nc, tp, sm, col_ap, ident, tag):
    """[128, 1] column (any partitions) -> [1, 128] row on partition 0."""
    ps = tp.tile([128, 128], F32, tag="tp")
    nc.tensor.transpose(ps[0:1, :], col_ap, ident)
    row = sm.tile([1, 128], F32, tag=tag)
    nc.scalar.copy(row, ps[0:1, 0:128])
    return row


def _row_to_col(nc, tp, sm, row_ap, one1, tag, scale=1.0):
    """[1, 128] row on partition 0 -> [128, 1] column, via PE transpose."""
    ps = tp.tile([128, 128], F32, tag="tp")
    nc.tensor.transpose(ps[:, 0:1], row_ap, one1)
    col = sm.tile([128, 1], F32, tag=tag)
    nc.scalar.activation(col, ps[:, 0:1], AF.Copy, bias=0.0, scale=scale)
    return col


def _dup_row(nc, sm, half_ap, tag):
    """[1, 64] -> [1, 128] duplicated halves."""
    row = sm.tile([1, 128], F32, tag=tag)
    nc.vector.tensor_copy(row[:, 0:64], half_ap)
    nc.vector.tensor_copy(row[:, 64:128], half_ap)
    return row


def _bn_combine(nc, tp, sm, ident, bnag, pfx):
    """Per-channel mean/var from per-partition bn_aggr [128, 2] output."""
    mrow = _col_to_row(nc, tp, sm, bnag[:, 0:1], ident, pfx + "mr")
    vrow = _col_to_row(nc, tp, sm, bnag[:, 1:2], ident, pfx + "vr")
    mu = sm.tile([1, 64], F32, tag=pfx + "mu")
    nc.vector.tensor_tensor(mu, mrow[:, 0:64], mrow[:, 64:128], op=ALU.add)
    nc.vector.tensor_scalar(mu, mu, 0.5, None, op0=ALU.mult)
    e2 = sm.tile([1, 128], F32, tag=pfx + "e2")
    nc.vector.tensor_tensor(e2, mrow, mrow, op=ALU.mult)
    nc.vector.tensor_tensor(e2, e2, vrow, op=ALU.add)
    e2h = sm.tile([1, 64], F32, tag=pfx + "e2h")
    nc.vector.tensor_tensor(e2h, e2[:, 0:64], e2[:, 64:128], op=ALU.add)
    nc.vector.tensor_scalar(e2h, e2h, 0.5, None, op0=ALU.mult)
    mq = sm.tile([1, 64], F32, tag=pfx + "mq")
    nc.vector.tensor_tensor(mq, mu, mu, op=ALU.mult)
    var = sm.tile([1, 64], F32, tag=pfx + "var")
    nc.vector.tensor_tensor(var, e2h, mq, op=ALU.subtract)
    return mu, var


def _chan_stats(nc, tp, sm, ident, ssum, ssq, pfx):
    """Per-channel mean/var from per-partition sums.
    ssum/ssq: [128, 1] per-(c,parity) sums of x and x^2 (8192 elems each).
    Returns (mu [1,64], var [1,64]) on partition 0."""
    sr = _col_to_row(nc, tp, sm, ssum, ident, pfx + "sr")
    qr = _col_to_row(nc, tp, sm, ssq, ident, pfx + "qr")
    mu = sm.tile([1, 64], F32, tag=pfx + "mu")
    nc.vector.tensor_tensor(mu, sr[:, 0:64], sr[:, 64:128], op=ALU.add)
    nc.vector.tensor_scalar(mu, mu, 1.0 / 16384.0, None, op0=ALU.mult)
    ex2 = sm.tile([1, 64], F32, tag=pfx + "ex2")
    nc.vector.tensor_tensor(ex2, qr[:, 0:64], qr[:, 64:128], op=ALU.add)
    nc.vector.tensor_scalar(ex2, ex2, 1.0 / 16384.0, None, op0=ALU.mult)
    mq = sm.tile([1, 64], F32, tag=pfx + "mq")
    nc.vector.tensor_tensor(mq, mu, mu, op=ALU.mult)
    var = sm.tile([1, 64], F32, tag=pfx + "var")
    nc.vector.tensor_tensor(var, ex2, mq, op=ALU.subtract)
    return mu, var


def build_kernel(nc: bass.Bass, tc: tile.TileContext, ctx):
    x = nc.dram_tensor("x", [G * C, H, W], F32, kind="ExternalInput").ap()
    w1 = nc.dram_tensor("w1", [G, C, C], F32, kind="ExternalInput").ap()
    b1 = nc.dram_tensor("b1", [G, C], F32, kind="ExternalInput").ap()
    w3 = nc.dram_tensor("w3", [G, C, C, 3, 3], F32, kind="ExternalInput").ap()
    b3 = nc.dram_tensor("b3", [G, C], F32, kind="ExternalInput").ap()
    gnw = nc.dram_tensor("gnw", [G, C], F32, kind="ExternalInput").ap()
    gnb = nc.dram_tensor("gnb", [G, C], F32, kind="ExternalInput").ap()
    y = nc.dram_tensor("y", [G * C, H, W], F32, kind="ExternalOutput").ap()

    big = ctx.enter_context(tc.tile_pool(name="big", bufs=1))
    wp = ctx.enter_context(tc.tile_pool(name="wp", bufs=1))
    sm = ctx.enter_context(tc.tile_pool(name="sm", bufs=1))
    pre = ctx.enter_context(tc.tile_pool(name="pre", bufs=1))
    sgp = ctx.enter_context(tc.tile_pool(name="sgp", bufs=3))
    sqp = ctx.enter_context(tc.tile_pool(name="sqp", bufs=2))
    sgw = ctx.enter_context(tc.tile_pool(name="sgw", bufs=3))
    pp = ctx.enter_context(tc.tile_pool(name="pp", bufs=3, space="PSUM"))
    pw = ctx.enter_context(tc.tile_pool(name="pw", bufs=3, space="PSUM"))
    tp = ctx.enter_context(tc.tile_pool(name="tp", bufs=1, space="PSUM"))
    pr = pp

    # ---------------- persistent big tiles ----------------
    t_feat = big.tile([128, 66, 130], F32)   # padded feature (conv input)
    t_gx = big.tile([128, 64, 128], F32)     # gated -> x1
    t_scr = big.tile([128, 64, 128], F32)    # x12 home
    t_pf = big.tile([128, 64, 128], F32)     # prefetched next-group input

    # ---------------- constants ----------------
    ident = wp.tile([128, 128], F32)
    make_identity(nc, ident)
    ones64 = wp.tile([1, 64], F32)
    nc.vector.memset(ones64, 1.0)
    one1 = ones64[0:1, 0:1]
    ones64r = wp.tile([1, 64], F32)
    nc.vector.tensor_copy(_r(ones64r), ones64)

    # zero halo rows and pad cols of feat once (conv reads them as f32r;
    # memset cannot emit f32r so round through a DVE copy)
    zcol = wp.tile([128, 1], F32)
    nc.vector.memset(zcol, 0.0)
    nc.vector.tensor_copy(_r(t_feat[:, 0, :]), zcol.broadcast_to((128, 130)))
    nc.vector.tensor_copy(_r(t_feat[:, 65, :]), zcol.broadcast_to((128, 130)))
    nc.vector.tensor_copy(
        _r(t_feat[:, :, 0:1]), zcol.unsqueeze(1).broadcast_to((128, 66, 1))
    )
    nc.vector.tensor_copy(
        _r(t_feat[:, :, 129:130]), zcol.unsqueeze(1).broadcast_to((128, 66, 1))
    )

    # ---------------- prepack params ----------------
    # w1: [G,C,C] (o,c) -> SBUF [64(o), G, 64(c)]
    w1raw = wp.tile([64, G, 64], F32)
    nc.sync.dma_start(out=w1raw, in_=w1.rearrange("g o c -> o g c"))
    # rows of small params on partition 0
    b1r = wp.tile([1, G, 64], F32)
    nc.sync.dma_start(out=b1r, in_=b1.rearrange("g c -> (g c)").unsqueeze(0))
    b3r = wp.tile([1, G, 64], F32)
    nc.sync.dma_start(out=b3r, in_=b3.rearrange("g c -> (g c)").unsqueeze(0))
    gwr = wp.tile([1, G, 64], F32)
    nc.sync.dma_start(out=gwr, in_=gnw.rearrange("g c -> (g c)").unsqueeze(0))
    gbr = wp.tile([1, G, 64], F32)
    nc.sync.dma_start(out=gbr, in_=gnb.rearrange("g c -> (g c)").unsqueeze(0))

    # transposed w1 (lhsT [c, o]), prescaled by 1/128 (pool means)
    w1s = wp.tile([64, G, 64], F32)
    # conv taps.  fp32r matmuls require PE tile column 0, so parity-1 taps
    # are M=128-padded (output cols 0:64 zero, 64:128 real) and parity-0
    # keeps M=64 at col 0.
    wstk = wp.tile([128, G, 3, 64], F32)      # par0 stacked (K=128)
    wsgl = wp.tile([128, G, 3, 64], F32)      # par0 singles (K=64 @ rows 64+)
    wstk1 = wp.tile([128, G, 3, 128], F32)    # par1 stacked, M-padded
    wsgl1 = wp.tile([64, G, 3, 128], F32)     # par1 singles, M-padded
    # zero the padded tiles once (rounding copies mark them f32r)
    zc2 = wp.tile([128, 1], F32)
    nc.vector.memset(zc2, 0.0)
    nc.vector.tensor_copy(
        _r(wstk1.rearrange("p g d m -> p (g d m)")),
        zc2.broadcast_to((128, G * 3 * 128)),
    )
    nc.vector.tensor_copy(
        _r(wsgl1.rearrange("p g d m -> p (g d m)")),
        zc2[0:64, :].broadcast_to((64, G * 3 * 128)),
    )
    # repl masks: [1, 128] rows selecting output partition halves
    maskA = wp.tile([1, 128], F32)
    nc.vector.memset(maskA[:, 0:64], 1.0)
    nc.vector.memset(maskA[:, 64:128], 0.0)
    maskAr = wp.tile([1, 128], F32)
    nc.vector.tensor_copy(_r(maskAr), maskA)
    maskBr = wp.tile([1, 128], F32)
    nc.vector.memset(maskA[:, 0:64], 0.0)
    nc.vector.memset(maskA[:, 64:128], 1.0)
    nc.vector.tensor_copy(_r(maskBr), maskA)
    # per-group vectors
    b1v = wp.tile([64, G], F32)     # conv1x1 bias per o
    v11 = wp.tile([128, G], F32)    # softmax(gnb) duplicated
    kv = wp.tile([128, G], F32)     # sigmoid(gnb) duplicated
    cb3 = wp.tile([1, G], F32)      # sum(x11 * b3)

    tc.strict_bb_all_engine_barrier()

    for g in range(G):
        pt = tp.tile([128, 128], F32, tag="tp")
        nc.tensor.transpose(pt[0:64, 0:64], w1raw[:, g, :], ident[0:64, 0:64])
        nc.scalar.activation(
            _r(w1s[:, g, :]), pt[0:64, 0:64], AF.Copy, bias=0.0, scale=1.0 / 128.0
        )
        w3raw = pre.tile([64, 64, 9], F32, tag="w3raw")
        nc.sync.dma_start(
            out=w3raw, in_=w3[g].rearrange("o c kh kw -> o c (kh kw)")
        )
        # conv taps: tap index t = ky*3 + kx  (ky = dy+1, kx = dx+1)
        # transpose each tap to [c, o], stage, then 6 grouped DMAs place them:
        # ky=1 -> stkE[0:64] + stkO[64:128]; ky=2 -> stkE[64:128] + sgl[0:64];
        # ky=0 -> stkO[0:64] + sgl[64:128]
        stage = pre.tile([64, 9, 64], F32, tag="tapstage")
        for tapidx in range(9):
            src = w3raw[:, :, tapidx]  # [64(o), 64(c)] strided
            ptt = tp.tile([128, 128], F32, tag="tp")
            pslice = ptt[0:64, 0:64]
            nc.tensor.transpose(pslice, src, ident[0:64, 0:64])
            nc.scalar.copy(stage[:, tapidx, :], pslice)
        nc.sync.dma_start(out=_r(wstk[0:64, g, :, :]), in_=_r(stage[:, 3:6, :]))
        nc.sync.dma_start(out=_r(wstk[64:128, g, :, :]), in_=_r(stage[:, 6:9, :]))
        nc.sync.dma_start(out=_r(wsgl[64:128, g, :, :]), in_=_r(stage[:, 0:3, :]))
        nc.sync.dma_start(
            out=_r(wstk1[0:64, g, :, 64:128]), in_=_r(stage[:, 0:3, :])
        )
        nc.sync.dma_start(
            out=_r(wstk1[64:128, g, :, 64:128]), in_=_r(stage[:, 3:6, :])
        )
        nc.sync.dma_start(
            out=_r(wsgl1[:, g, :, 64:128]), in_=_r(stage[:, 6:9, :])
        )
        # b1 column
        ptb = tp.tile([128, 128], F32, tag="tp")
        nc.tensor.transpose(ptb[0:64, 0:1], b1r[:, g, :], one1)
        nc.scalar.copy(b1v[:, g : g + 1], ptb[0:64, 0:1])
        # x11 = softmax(gnb[g]); k = sigmoid(gnb[g])
        x11 = _sigmoid_softmax(nc, sm, gbr[:, g, :], 64)
        x11d = _dup_row(nc, sm, x11, "x11d")
        ptv = tp.tile([128, 128], F32, tag="tp")
        nc.tensor.transpose(ptv[:, 0:1], x11d, one1)
        nc.scalar.copy(_r(v11[:, g : g + 1]), ptv[:, 0:1])
        krow = sm.tile([1, 64], F32, tag="krow")
        nc.scalar.activation(krow, gbr[:, g, :], AF.Sigmoid)
        krd = _dup_row(nc, sm, krow, "krd")
        ptk = tp.tile([128, 128], F32, tag="tp")
        nc.tensor.transpose(ptk[:, 0:1], krd, one1)
        nc.scalar.copy(kv[:, g : g + 1], ptk[:, 0:1])
        # cb3 = sum(x11 * b3)
        xb = sm.tile([1, 64], F32, tag="xb")
        nc.vector.tensor_tensor(xb, x11, b3r[:, g, :], op=ALU.mult)
        nc.vector.tensor_reduce(cb3[:, g : g + 1], xb, axis=AX.X, op=ALU.add)

    tc.strict_bb_all_engine_barrier()

    # ---------------- input DMA ----------------
    def dma_in(g, dst_even, dst_odd, f32r=False):
        gc0 = g * C
        cast = _r if f32r else (lambda ap: ap)
        nc.sync.dma_start(out=cast(dst_even), in_=cast(x[gc0 : gc0 + 64, 0:128:2, :]))
        nc.sync.dma_start(out=cast(dst_odd), in_=cast(x[gc0 : gc0 + 64, 1:128:2, :]))

    # group 0 load is chunked so the g0 stats chain starts immediately
    for k in range(8):
        nc.sync.dma_start(
            out=_r(t_feat[0:64, 1 + 8 * k : 9 + 8 * k, 1:129]),
            in_=_r(x[0:64, 16 * k : 16 * k + 16 : 2, :]),
        )
        nc.sync.dma_start(
            out=_r(t_feat[64:128, 1 + 8 * k : 9 + 8 * k, 1:129]),
            in_=_r(x[0:64, 16 * k + 1 : 16 * k + 16 : 2, :]),
        )
    if G > 1:
        dma_in(1, t_pf[0:64, :, :], t_pf[64:128, :, :])

    feat_re = t_feat[:, 1:65, 1:129]  # real region [128, 64, 128]

    # ================= group loop =================
    for g in range(G):
        if g > 0:
            # feat += x_g, chunked so downstream per-chunk consumers stream
            for ci in range(NCHUNK):
                jb = 4 * ci
                nc.gpsimd.tensor_tensor(
                    _r(t_feat[:, 1 + jb : 5 + jb, 1:129]),
                    t_feat[:, 1 + jb : 5 + jb, 1:129],
                    t_pf[:, jb : jb + 4, :],
                    op=ALU.add,
                )
            if g + 1 < G:
                dma_in(g + 1, t_pf[0:64, :, :], t_pf[64:128, :, :])

        # ---- chunked pooled row sums + feat square sums ----
        # xh chunks (DVE) and fsq chunks (Act) stream behind the adds, so
        # feat stats are ready ~1 chunk after the last add.
        xh = sm.tile([128, 64], F32, tag="xh")     # row sums (over w)
        fsq_c = sm.tile([128, NCHUNK], F32, tag="fsq_c")
        for ci in range(NCHUNK):
            jb = 4 * ci
            nc.vector.tensor_reduce(
                xh[:, ci * 4 : ci * 4 + 4],
                t_feat[:, 1 + jb : 5 + jb, 1:129],
                axis=AX.X,
                op=ALU.add,
            )
            sqd = sqp.tile([128, 4, 128], F32, tag="sqd")
            if ci % 2 == 0:
                nc.scalar.activation(
                    sqd, t_feat[:, 1 + jb : 5 + jb, 1:129], AF.Square,
                    accum_out=fsq_c[:, ci : ci + 1],
                )
            else:
                nc.vector.scalar_tensor_tensor(
                    out=sqd,
                    in0=t_feat[:, 1 + jb : 5 + jb, 1:129],
                    scalar=1.0,
                    in1=t_feat[:, 1 + jb : 5 + jb, 1:129],
                    op0=ALU.mult,
                    op1=ALU.mult,
                    accum_out=fsq_c[:, ci : ci + 1],
                )
        fsum = sm.tile([128, 1], F32, tag="fsum")
        nc.vector.tensor_reduce(fsum, xh, axis=AX.X, op=ALU.add)
        fsq = sm.tile([128, 1], F32, tag="fsq")
        nc.vector.tensor_reduce(fsq, fsq_c, axis=AX.X, op=ALU.add)

        # cat_h: xh parity interleave (even rows -> cols 0:128:2, odd -> 1:)
        cat_h = sm.tile([64, 128], F32, tag="cat_h")
        nc.sync.dma_start(
            out=_r(cat_h.rearrange("p (h two) -> p h two", two=2)[:, :, 0:1]),
            in_=_r(xh[0:64, :].unsqueeze(2)),
        )
        nc.sync.dma_start(
            out=_r(cat_h.rearrange("p (h two) -> p h two", two=2)[:, :, 1:2]),
            in_=_r(xh[64:128, :].unsqueeze(2)),
        )

        sx12 = sm.tile([128, NCHUNK], F32, tag="sx12")
        sgc_tiles = {}

        # Deferred emission slots, keyed by the conv chunk index after which
        # they are emitted (positions the ops inside each engine queue).
        state = {}

        def slot_stats1():
            muf, varf = _chan_stats(nc, tp, sm, ident, fsum, fsq, "f")
            state["rfr"] = _rsqrt(nc, sm, varf, 64, "rf")
            nmf = sm.tile([1, 64], F32, tag="nmf")
            nc.vector.tensor_tensor(nmf, muf, state["rfr"], op=ALU.mult)
            brow = sm.tile([1, 128], F32, tag="brow")
            nc.vector.tensor_scalar(brow[:, 0:64], nmf, -1.0, None, op0=ALU.mult)
            nc.vector.tensor_scalar(brow[:, 64:128], nmf, -1.0, None, op0=ALU.mult)
            state["brow"] = brow

        def slot_stats2():
            srow = _dup_row(nc, sm, state["rfr"], "srow")
            state["rfv"] = _row_to_col(nc, tp, sm, srow, one1, "rfv")
            state["bfv"] = _row_to_col(nc, tp, sm, state["brow"], one1, "bfv")

        def slot_convh():
            phw = tp.tile([64, 128], F32, tag="phw")
            nc.tensor.matmul(phw, _r(w1s[:, g, :]), _r(cat_h), start=True, stop=True)
            sig_h = sm.tile([64, 128], F32, tag="sig_h")
            nc.scalar.activation(
                sig_h, phw, AF.Sigmoid, bias=b1v[:, g : g + 1], scale=1.0
            )
            sh_eo = sm.tile([128, 64], F32, tag="sh_eo")
            nc.sync.dma_start(
                out=sh_eo[0:64, :],
                in_=sig_h.rearrange("p (h two) -> p h two", two=2)[:, :, 0],
            )
            nc.sync.dma_start(
                out=sh_eo[64:128, :],
                in_=sig_h.rearrange("p (h two) -> p h two", two=2)[:, :, 1],
            )
            state["sh_eo"] = sh_eo

        def slot_convw():
            phw = tp.tile([64, 128], F32, tag="phw")
            nc.tensor.matmul(
                phw, _r(w1s[:, g, :]), _r(state["cat_w"]), start=True, stop=True
            )
            sig_w = sm.tile([64, 128], F32, tag="sig_w")
            nc.scalar.activation(
                sig_w, phw, AF.Sigmoid, bias=b1v[:, g : g + 1], scale=1.0
            )
            sw_eo = sm.tile([128, 128], F32, tag="sw_eo")
            nc.sync.dma_start(out=sw_eo[0:64, :], in_=sig_w)
            nc.sync.dma_start(out=sw_eo[64:128, :], in_=sig_w)
            state["sw_eo"] = sw_eo

        if g == 0:
            slots = {3: slot_stats1, 5: slot_stats2, 7: slot_convh, 13: slot_convw}
        else:
            # adds/xh/bn stream during the previous group's weights phase,
            # so stats can be emitted at the head without stalling PE
            slots = {0: slot_stats1, 1: slot_stats2, 2: slot_convh, 10: slot_convw}

        # ---- conv3x3 stream (PE) + Sg chunks (Act) + x12 evict (Pool) ----
        # Sg/evict emission lags conv emission until feat stats (rfv/bfv)
        # have been emitted by the ci==5 slot.
        # gating chain (Pool) + gated bn stats (DVE) lag the conv stream by
        # GLAG chunks so they interleave with the evicts instead of
        # serializing after them (x1 folding means only stats are needed).
        bnout = sm.tile([128, NCHUNK, 6], F32, tag="bnout")
        # xw (column sums over rows) is chunked and interleaved with the
        # evict stream so the monolithic reduce does not block the DVE
        # queue at the conv tail
        xw = sm.tile([128, 128], F32, tag="xw")
        xw_n = [0]

        def emit_xw_chunk():
            c = xw_n[0]
            nc.vector.tensor_reduce(
                xw[:, c * 16 : (c + 1) * 16],
                feat_re.rearrange("p j w -> p w j")[:, c * 16 : (c + 1) * 16, :],
                axis=AX.X,
                op=ALU.add,
            )
            xw_n[0] += 1
            if xw_n[0] == 8:
                xwhi = sm.tile([64, 128], F32, tag="xwhi")
                nc.sync.dma_start(out=xwhi, in_=xw[64:128, :])
                cat_w = sm.tile([64, 128], F32, tag="cat_w")
                nc.vector.tensor_tensor(
                    _r(cat_w), xw[0:64, :], xwhi, op=ALU.add
                )
                state["cat_w"] = cat_w

        pend = []

        def emit_g1(ci):
            jb = 4 * ci
            nc.gpsimd.tensor_tensor(
                _r(t_gx[:, jb : jb + 4, :]),
                t_feat[:, 1 + jb : 5 + jb, 1:129],
                state["sh_eo"][:, jb : jb + 4].unsqueeze(2).broadcast_to(
                    (128, 4, 128)
                ),
                op=ALU.mult,
            )

        def emit_g2bn(ci):
            jb = 4 * ci
            gx_c = t_gx[:, jb : jb + 4, :]
            nc.gpsimd.tensor_tensor(
                _r(gx_c),
                gx_c,
                state["sw_eo"].unsqueeze(1).broadcast_to((128, 4, 128)),
                op=ALU.mult,
            )
            nc.vector.bn_stats(
                bnout[:, ci, :], gx_c.rearrange("p a w -> p (a w)")
            )

        def emit_sg_evict(ci, pcs):
            # pcs is None for chunks already copied to t_scr (split evict)
            jb = 4 * ci
            sgc = sgp.tile([128, 4, 128], F32, tag="sgc")
            nc.scalar.activation(
                sgc, t_feat[:, 1 + jb : 5 + jb, 1:129], AF.Sigmoid,
                bias=state["bfv"], scale=state["rfv"],
            )
            scr_c = t_scr[:, jb : jb + 4, :]
            x2_src = (
                scr_c if pcs is None else pcs.rearrange("p (a b) -> p a b", a=4)
            )
            # x12 = x2 + k * Sg   (+ accumulate row-sums for x21)
            nc.vector.scalar_tensor_tensor(
                out=_r(scr_c),
                in0=sgc,
                scalar=kv[:, g : g + 1],
                in1=x2_src,
                op0=ALU.mult,
                op1=ALU.add,
                accum_out=sx12[:, ci : ci + 1],
            )

        for ci in range(NCHUNK):
            jb = 4 * ci
            if ci in slots:
                slots[ci]()
            # fp32r requires PE tile col 0, so parity 1 uses an M=128 group
            # (weight cols 0:64 zero) into its own PSUM tile and parity 0
            # an M=64 group into another.
            pc = pp.tile([128, 512], F32, tag="pconv1")
            for dx in range(3):
                nc.tensor.matmul(
                    pc,
                    _r(wstk1[:, g, dx, :]),
                    _r(t_feat[:, 1 + jb : 5 + jb, dx : dx + 128]),
                    start=(dx == 0),
                    stop=False,
                    tile_position=(0, 0),
                )
            for dx in range(3):
                nc.tensor.matmul(
                    pc,
                    _r(wsgl1[:, g, dx, :]),
                    _r(t_feat[0:64, 2 + jb : 6 + jb, dx : dx + 128]),
                    start=False,
                    stop=(dx == 2),
                    tile_position=(0, 0),
                )
            for dx in range(3):
                nc.tensor.matmul(
                    pc[0:64, :],
                    _r(wstk[:, g, dx, :]),
                    _r(t_feat[:, 1 + jb : 5 + jb, dx : dx + 128]),
                    start=(dx == 0),
                    stop=False,
                    tile_position=(0, 0),
                )
            for dx in range(3):
                nc.tensor.matmul(
                    pc[0:64, :],
                    _r(wsgl[64:128, g, dx, :]),
                    _r(t_feat[64:128, jb : 4 + jb, dx : dx + 128]),
                    start=False,
                    stop=(dx == 2),
                    tile_position=(64, 0),
                )
            pend.append((ci, pc))
            if "rfv" in state:
                for pci, ppcs in pend:
                    emit_sg_evict(pci, ppcs)
                pend = []
                if xw_n[0] < 8:
                    emit_xw_chunk()
        for ci in range(NCHUNK):
            emit_g1(ci)
        for ci in range(NCHUNK):
            emit_g2bn(ci)

        # gating chain was emitted interleaved with the conv stream (lag
        # GLAG chunks); aggregate the gated bn stats here
        bnag = sm.tile([128, 2], F32, tag="bnag")
        nc.vector.bn_aggr(bnag, bnout)
        mug, varg = _bn_combine(nc, tp, sm, ident, bnag, "g")
        rgr = _rsqrt(nc, sm, varg, 64, "rg")
        s1 = sm.tile([1, 64], F32, tag="s1")
        nc.vector.tensor_tensor(s1, gwr[:, g, :], rgr, op=ALU.mult)
        nmg = sm.tile([1, 64], F32, tag="nmg")
        nc.vector.tensor_tensor(nmg, mug, s1, op=ALU.mult)
        bx1 = sm.tile([1, 64], F32, tag="bx1")
        nc.vector.scalar_tensor_tensor(
            bx1, nmg, -1.0, gbr[:, g, :], op0=ALU.mult, op1=ALU.add
        )

        # ---- x21 = softmax_c(mean(x12) + b3) ----
        sxr = sm.tile([128, 1], F32, tag="sxr")
        nc.vector.tensor_reduce(sxr, sx12, axis=AX.X, op=ALU.add)
        sxrow = _col_to_row(nc, tp, sm, sxr, ident, "sxrow")
        sxf = sm.tile([1, 64], F32, tag="sxf")
        nc.vector.tensor_tensor(sxf, sxrow[:, 0:64], sxrow[:, 64:128], op=ALU.add)
        x21in = sm.tile([1, 64], F32, tag="x21in")
        nc.vector.scalar_tensor_tensor(
            x21in, sxf, 1.0 / 16384.0, b3r[:, g, :], op0=ALU.mult, op1=ALU.add
        )
        x21 = _sigmoid_softmax(nc, sm, x21in, 64)
        # v21' = x21 * s1 (folded GN scale); c21 = sum(x21 * bx1)
        v21r = sm.tile([1, 64], F32, tag="v21r")
        nc.vector.tensor_tensor(v21r, x21, s1, op=ALU.mult)
        xbx = sm.tile([1, 64], F32, tag="xbx")
        nc.vector.tensor_tensor(xbx, x21, bx1, op=ALU.mult)
        c21 = sm.tile([1, 1], F32, tag="c21")
        nc.vector.tensor_reduce(c21, xbx, axis=AX.X, op=ALU.add)
        # total sigmoid bias = cb3[g] + c21
        btot = sm.tile([1, 1], F32, tag="btot")
        nc.vector.tensor_tensor(btot, c21, cb3[:, g : g + 1], op=ALU.add)
        # v21' column via transpose
        v21d = _dup_row(nc, sm, v21r, "v21d")
        ptv21 = tp.tile([128, 128], F32, tag="tp")
        nc.tensor.transpose(ptv21[:, 0:1], v21d, one1)
        v21 = sm.tile([128, 1], F32, tag="v21")
        nc.scalar.copy(_r(v21), ptv21[:, 0:1])

        # ---- weights = x11 @ x12 + x21 @ x1 ; out = feat * sigmoid(weights)
        # raw weights row -> SBUF (Pool) -> broadcast (PE) -> one sigmoid per
        # chunk over all 128 partitions (Act) -> multiply (DVE/Pool)
        for ci in range(NCHUNK):
            jb = 4 * ci
            prt = pr.tile([128, 512], F32, tag="pconv1")
            sw_cs = []
            for par in range(2):
                pbase = 64 * par
                chunk = (slice(pbase, pbase + 64), slice(jb, jb + 4), slice(None))
                pwt = pw.tile([1, 512], F32, tag="pw")
                nc.tensor.matmul(
                    pwt,
                    _r(v11[pbase : pbase + 64, g : g + 1]),
                    _r(t_scr[chunk[0], chunk[1], :]),
                    start=True,
                    stop=False,
                    tile_position=(pbase, 0),
                )
                nc.tensor.matmul(
                    pwt,
                    _r(v21[pbase : pbase + 64, :]),
                    _r(t_gx[chunk[0], chunk[1], :]),
                    start=False,
                    stop=True,
                    tile_position=(pbase, 0),
                )
                sw_c = sgw.tile([1, 512], F32, tag="swrow")
                nc.scalar.activation(
                    _r(sw_c), pwt, AF.Sigmoid, bias=btot, scale=1.0
                )
                sw_cs.append(sw_c)
            # broadcast both parity rows into [128, 512] in one M=128
            # masked group (fp32r requires PE tile col 0)
            nc.tensor.matmul(
                prt, _r(maskAr), _r(sw_cs[0]), start=True, stop=False,
                tile_position=(0, 0),
            )
            nc.tensor.matmul(
                prt, _r(maskBr), _r(sw_cs[1]), start=False, stop=True,
                tile_position=(0, 0),
            )
            nc.vector.tensor_tensor(
                _r(t_feat[:, 1 + jb : 5 + jb, 1:129]),
                t_feat[:, 1 + jb : 5 + jb, 1:129],
                prt.rearrange("p (a b) -> p a b", a=4),
                op=ALU.mult,
            )
            # chunked output DMA: releases this chunk of t_feat for the
            # next group's cascade add immediately
            gc0 = g * C
            nc.sync.dma_start(
                out=y[gc0 : gc0 + 64, 2 * jb : 2 * jb + 8 : 2, :],
                in_=t_feat[0:64, 1 + jb : 5 + jb, 1:129],
            )
            nc.gpsimd.dma_start(
                out=y[gc0 : gc0 + 64, 2 * jb + 1 : 2 * jb + 8 : 2, :],
                in_=t_feat[64:128, 1 + jb : 5 + jb, 1:129],
            )

    return nc


_CACHE = {}


def _get_nc(split=True):
    if "nc" not in _CACHE:
        from contextlib import ExitStack

        nc = bacc.Bacc(
            "TRN2", target_bir_lowering=False, debug=False, num_devices=8
        )
        with tile.TileContext(nc) as tc:
            with ExitStack() as ctx:
                build_kernel(nc, tc, ctx)
        nc.compile()
        _CACHE["nc"] = nc
    return _CACHE["nc"]


def kernel(x, w1, b1, w3, b3, gnw, gnb):
    nc = _get_nc()
    from concourse.bass_utils import run_bass_kernel_spmd

    x = np.ascontiguousarray(np.asarray(x, dtype=np.float32))
    params = {
        "w1": np.ascontiguousarray(np.asarray(w1, np.float32)),
        "b1": np.ascontiguousarray(np.asarray(b1, np.float32)),
        "w3": np.ascontiguousarray(np.asarray(w3, np.float32)),
        "b3": np.ascontiguousarray(np.asarray(b3, np.float32)),
        "gnw": np.ascontiguousarray(np.asarray(gnw, np.float32)),
        "gnb": np.ascontiguousarray(np.asarray(gnb, np.float32)),
    }
    in_maps = [dict(params, x=np.ascontiguousarray(x[i])) for i in range(8)]
    res = run_bass_kernel_spmd(nc, in_maps, list(range(8)))
    out = np.stack([res.results[i]["y"] for i in range(8)], axis=0)
    return out

